# revision 12
# baseline (speedup 1.0000x reference)
"""Bass/Trainium2 kernel for nn_DisentangleLayer (FactorGCN-style GNN layer).

Math (per reference):
  h    = x @ W_lin + b_lin                    [N, 256]
  h_em = x @ emb_w + emb_b                    [N, 64]
  s_src = h @ a_src.T ; s_dst = h @ a_dst.T   [N, 4]    (att_w = [a_src | a_dst])
  e    = sigmoid(s_src[src] + s_dst[dst] + att_b)       [E, 4]
  ev   = exp(e - max(e))     (max subtraction cancels in the normalized
                              ratio below; sigmoid output is bounded so no
                              overflow risk -> we skip it)
  denom = segsum_src(ev)                       [N, 4]
  out[n, 64l:64l+64] = segsum_src(ev_l * h_em[dst]) / denom[n, l]

Strategy:
  * Host shards edges by src-range across 8 cores (each core owns 12500
    nodes' outputs; no cross-core reduction needed).
  * Per core, edges are sorted by src and mapped to dense ranks; ranks are
    grouped into 128-node windows; each window's edge list is padded to a
    fixed number of 128-edge tiles (uniform across cores -> one compiled
    program).
  * Device: phase A computes a packed per-node table
    [h_em(64) | s_dst+att_b(4) | s_src(4) | pad] (f32, 512B rows) for all
    N nodes (replicated per core).  Edge phase gathers table rows by dst
    via indirect DMA, forms per-tile one-hot matrices S (edges x ranks)
    and S^T on-chip, and uses TensorE matmuls for the per-window segment
    sums of [ev*h_em | ev]; normalization by denom happens per window.
"""

import math
import numpy as np
from contextlib import ExitStack

P = 128
CORES = 8
IN_F = 256
D_EM = 64
L = 4

_PATCHED = False


def _apply_tile_patch():
    """walrus in this env rejects >1 sem-wait on one instruction; split the
    TileContext exit-drain waits across single-wait nops."""
    global _PATCHED
    if _PATCHED:
        return
    _PATCHED = True
    import concourse.tile as tile_mod
    import concourse.mybir as mybir
    from concourse.vector_clock import ScopedClock

    def _drain_and_barrier(self, tick_clock, wait_clock):
        nop = self.nc.sync.nop()
        wait_clock.add_sem_waits(nop.ins, ScopedClock({None: tick_clock.global_clock}))
        si = nop.ins.sync_info
        waits = list(si.on_wait) if si is not None else []
        if len(waits) > 1:
            si.on_wait = waits[:1]
            nop.ins.sync_info = si
            for i in range(1, len(waits)):
                extra = self.nc.sync.nop()
                extra.ins.sync_info = mybir.SyncInfo(
                    on_wait=waits[i : i + 1], on_update=[]
                )
        self.nc.sync.drain()
        self.nc.all_engine_barrier()
        assert self.sems is not None
        popped = self.nc._tile_sem_poison_stack.pop()
        assert popped is self._sem_poison
        self.nc.clear_and_free_semaphores(list(self.sems.allocated().values()))
        self.nc.all_engine_barrier()

    tile_mod.TileContext._drain_and_barrier = _drain_and_barrier


# ----------------------------------------------------------------------------
# host-side sharding / stream building
# ----------------------------------------------------------------------------

def _host_prep(src, dst, n_nodes, n_cores):
    """Shard edges by src range, sort by src, build per-core device streams.

    Returns (cfg, per_core) where per_core[c] is a dict of numpy arrays and
    cfg holds the uniform shape parameters.
    """
    NV = n_nodes // n_cores
    NW = (NV + P - 1) // P
    src = np.asarray(src)
    dst = np.asarray(dst)

    cores = []
    for c in range(n_cores):
        lo = c * NV
        sel = (src >= lo) & (src < lo + NV)
        es = src[sel].astype(np.int64) - lo
        ed = dst[sel].astype(np.int64)
        order = np.argsort(es, kind="stable")
        es = es[order]
        ed = ed[order]
        u, counts = np.unique(es, return_counts=True)
        K = len(u)
        ranks = np.repeat(np.arange(K, dtype=np.int64), counts)
        w = ranks // P
        cnt_w = np.bincount(w, minlength=NW)
        cores.append((ed, u, K, ranks, w, cnt_w))

    T_w = 1
    for (_, _, _, _, _, cnt_w) in cores:
        T_w = max(T_w, int(math.ceil(cnt_w.max() / P)))

    per_core = []
    for c in range(n_cores):
        ed, u, K, ranks, w, cnt_w = cores[c]
        lo = c * NV
        nslot = T_w * P
        slot_rank = np.full((NW, nslot), -1.0, np.float32)
        slot_dst = np.zeros((NW, nslot), np.int32)
        offs = np.concatenate([[0], np.cumsum(cnt_w)])
        pos = np.arange(len(ed)) - offs[w]
        slot_rank[w, pos] = (ranks % P).astype(np.float32)
        slot_dst[w, pos] = ed.astype(np.int32)

        # device layouts: [128, NW*T_w] with slot (w, i, p) -> [p, w*T_w + i]
        rank_col = (
            slot_rank.reshape(NW, T_w, P).transpose(2, 0, 1).reshape(P, NW * T_w)
        )
        didx = slot_dst.reshape(NW, T_w, P).transpose(2, 0, 1).reshape(P, NW * T_w)

        # ST one-hot bytes: stb[n, (w, i, e)] == 1 iff rank of slot
        # (w, i, e) == n.   (e is the partition index of the edge.)
        stb = np.zeros((P, NW, T_w, P), np.uint8)
        sr = slot_rank.reshape(NW, T_w, P)
        wv, iv, evi = np.nonzero(sr >= 0)
        nv = sr[wv, iv, evi].astype(np.int64)
        stb[nv, wv, iv, evi] = 1
        stb = stb.reshape(P, NW * T_w * P)

        u_pad = np.zeros(NW * P, np.int32)
        u_pad[:K] = (u + lo).astype(np.int32)
        uidx = u_pad.reshape(NW, P).T.copy()  # [128, NW]

        per_core.append(
            dict(didx=didx, rankc=rank_col, stb=stb, uidx=uidx, u=u, K=K)
        )

    cfg = dict(NV=NV, NW=NW, T_w=T_w)
    return cfg, per_core


# ----------------------------------------------------------------------------
# device program
# ----------------------------------------------------------------------------

def _split_multi_waits(nc):
    """This env's walrus accepts at most ONE sync-wait command per
    instruction.  Move extra waits onto single-wait nops inserted just
    before the instruction on the same engine (same sequencer => identical
    semantics)."""
    import concourse.mybir as mybir

    cnt = 0
    for f in nc.m.functions:
        for blk in f.blocks:
            insts = blk.instructions
            out = []
            changed = False
            for ins in insts:
                si = ins.sync_info
                waits = list(si.on_wait) if si is not None else []
                if len(waits) > 1:
                    changed = True
                    for w in waits[:-1]:
                        cnt += 1
                        nop = mybir.InstNoOp(
                            name=f"wsplit_{cnt}", ins=[], outs=[]
                        )
                        nop.engine = ins.engine
                        nop.sync_info = mybir.SyncInfo(on_wait=[w], on_update=[])
                        out.append(nop)
                    si.on_wait = waits[-1:]
                    ins.sync_info = si
                out.append(ins)
            if changed:
                blk.instructions = out
    return cnt


def _build_nc(N, NW, T_w, TC=16, split_waits=True):
    _apply_tile_patch()
    import concourse.bass as bass
    import concourse.mybir as mybir
    import concourse.tile as tile
    from concourse.masks import make_identity

    f32 = mybir.dt.float32
    i32 = mybir.dt.int32
    u8 = mybir.dt.uint8
    AF = mybir.ActivationFunctionType
    OP = mybir.AluOpType
    IOOA = bass.IndirectOffsetOnAxis

    nc = bass.Bass()
    x_d = nc.declare_dram_parameter("x", [N, IN_F], f32, isOutput=False)
    wl_d = nc.declare_dram_parameter("wl", [IN_F, IN_F], f32, isOutput=False)
    aw_d = nc.declare_dram_parameter("aw", [L, 2 * IN_F], f32, isOutput=False)
    ew_d = nc.declare_dram_parameter("ew", [IN_F, D_EM], f32, isOutput=False)
    bl_d = nc.declare_dram_parameter("bl", [IN_F, 1], f32, isOutput=False)
    embb_d = nc.declare_dram_parameter("embb", [1, D_EM], f32, isOutput=False)
    attb_d = nc.declare_dram_parameter("attb", [1, L], f32, isOutput=False)
    didx_d = nc.declare_dram_parameter("didx", [P, NW * T_w], i32, isOutput=False)
    rankc_d = nc.declare_dram_parameter("rankc", [P, NW * T_w], f32, isOutput=False)
    stb_d = nc.declare_dram_parameter("stb", [P, NW * T_w * P], u8, isOutput=False)
    uidx_d = nc.declare_dram_parameter("uidx", [P, NW], i32, isOutput=False)
    iota_d = nc.declare_dram_parameter("iota_mat", [P, P], f32, isOutput=False)
    descale_d = nc.declare_dram_parameter("descale", [P, 1], f32, isOutput=False)
    out_d = nc.declare_dram_parameter("out", [NW * P, 4 * D_EM], f32, isOutput=True)

    tbl = nc.dram_tensor("tbl", [N, P], f32)  # [h_em(64)|s_dst+attb(4)|s_src(4)|0]

    ntileA = (N + P - 1) // P
    chunks = []
    k0 = 0
    while k0 < T_w:
        chunks.append((k0, min(TC, T_w - k0)))
        k0 += TC

    with ExitStack() as ctx:
        tc = ctx.enter_context(tile.TileContext(nc))
        const = ctx.enter_context(tc.tile_pool(name="const", bufs=1))

        ident = const.tile([P, P], f32)
        make_identity(nc, ident[:])
        iota = const.tile([P, P], f32)
        nc.sync.dma_start(out=iota[:], in_=iota_d[:])
        descale = const.tile([P, 1], f32)
        nc.sync.dma_start(out=descale[:], in_=descale_d[:])

        # ---- fold weights: Wp[ic] = [emb_w | W@a_dst.T | W@a_src.T | 0] ----
        WT = [[const.tile([P, P], f32, name=f"WT_{j}_{i}") for i in range(2)] for j in range(2)]
        adT = [const.tile([P, L], f32, name=f"adT_{j}") for j in range(2)]
        asT = [const.tile([P, L], f32, name=f"asT_{j}") for j in range(2)]
        blT = [const.tile([P, 1], f32, name=f"blT_{j}") for j in range(2)]
        Wp = [const.tile([P, P], f32, name=f"Wp_{i}") for i in range(2)]
        bias_row = const.tile([1, P], f32)
        ones1 = const.tile([1, P], f32)
        bias_bc = const.tile([P, P], f32)
        attb_sb = const.tile([1, L], f32)

        with (
            tc.tile_pool(name="setup_sb", bufs=2) as ssb,
            tc.tile_pool(name="setup_ps", bufs=2, space="PSUM") as sps,
        ):
            for jc in range(2):
                nc.sync.dma_start(
                    out=adT[jc][:],
                    in_=aw_d[:, IN_F + jc * P : IN_F + (jc + 1) * P].transpose([1, 0]),
                )
                nc.sync.dma_start(
                    out=asT[jc][:],
                    in_=aw_d[:, jc * P : (jc + 1) * P].transpose([1, 0]),
                )
                nc.sync.dma_start(out=blT[jc][:], in_=bl_d[jc * P : (jc + 1) * P, :])
                for ic in range(2):
                    wt = ssb.tile([P, P], f32)
                    nc.sync.dma_start(
                        out=wt[:],
                        in_=wl_d[ic * P : (ic + 1) * P, jc * P : (jc + 1) * P],
                    )
                    tp = sps.tile([P, P], f32, space="PSUM")
                    nc.tensor.transpose(out=tp[:], in_=wt[:], identity=ident[:])
                    nc.vector.tensor_copy(out=WT[jc][ic][:], in_=tp[:])

            for ic in range(2):
                nc.gpsimd.memset(Wp[ic][:], 0)
                nc.sync.dma_start(
                    out=Wp[ic][:, 0:D_EM], in_=ew_d[ic * P : (ic + 1) * P, :]
                )
                wd_ps = sps.tile([P, 2 * L], f32, space="PSUM")
                for t, rhs_t in ((0, adT), (1, asT)):
                    for jc in range(2):
                        nc.tensor.matmul(
                            out=wd_ps[:, t * L : (t + 1) * L],
                            lhsT=WT[jc][ic][:],
                            rhs=rhs_t[jc][:],
                            start=(jc == 0),
                            stop=(jc == 1),
                        )
                nc.vector.tensor_copy(
                    out=Wp[ic][:, D_EM : D_EM + 2 * L], in_=wd_ps[:]
                )

            bias_ps = sps.tile([1, 2 * L], f32, space="PSUM")
            for t, rhs_t in ((0, adT), (1, asT)):
                for jc in range(2):
                    nc.tensor.matmul(
                        out=bias_ps[:, t * L : (t + 1) * L],
                        lhsT=blT[jc][:],
                        rhs=rhs_t[jc][:],
                        start=(jc == 0),
                        stop=(jc == 1),
                    )
            nc.gpsimd.memset(bias_row[:], 0)
            nc.sync.dma_start(out=bias_row[:, 0:D_EM], in_=embb_d[:])
            nc.sync.dma_start(out=attb_sb[:], in_=attb_d[:])
            nc.vector.tensor_tensor(
                out=bias_row[:, D_EM : D_EM + L],
                in0=bias_ps[:, 0:L],
                in1=attb_sb[:],
                op=OP.add,
            )
            nc.vector.tensor_copy(
                out=bias_row[:, D_EM + L : D_EM + 2 * L], in_=bias_ps[:, L : 2 * L]
            )
            # broadcast bias_row across partitions via K=1 matmul
            nc.gpsimd.memset(ones1[:], 1.0)
            bb_ps = sps.tile([P, P], f32, space="PSUM")
            nc.tensor.matmul(
                out=bb_ps[:], lhsT=ones1[:], rhs=bias_row[:], start=True, stop=True
            )
            nc.vector.tensor_copy(out=bias_bc[:], in_=bb_ps[:])

        # ---- phase A: build tbl[N, 128] ----
        with (
            tc.tile_pool(name="xa", bufs=3) as xa,
            tc.tile_pool(name="xt", bufs=3) as xtp,
            tc.tile_pool(name="stg", bufs=3) as stg,
            tc.tile_pool(name="psT", bufs=2, space="PSUM") as psT,
            tc.tile_pool(name="psM", bufs=2, space="PSUM") as psM,
        ):
            for i in range(ntileA):
                r0 = i * P
                pp = min(P, N - r0)
                xt = xa.tile([P, IN_F], f32)
                nc.sync.dma_start(out=xt[:pp, :], in_=x_d[r0 : r0 + pp, :])
                xTs = []
                for jc in range(2):
                    tp = psT.tile([P, P], f32, space="PSUM")
                    nc.tensor.transpose(
                        out=tp[:, :pp],
                        in_=xt[:pp, jc * P : (jc + 1) * P],
                        identity=ident[:pp, :pp],
                    )
                    xT = xtp.tile([P, P], f32)
                    nc.scalar.copy(out=xT[:, :pp], in_=tp[:, :pp])
                    xTs.append(xT)
                tab_ps = psM.tile([P, P], f32, space="PSUM")
                for jc in range(2):
                    nc.tensor.matmul(
                        out=tab_ps[:pp, :],
                        lhsT=xTs[jc][:, :pp],
                        rhs=Wp[jc][:],
                        start=(jc == 0),
                        stop=(jc == 1),
                    )
                st = stg.tile([P, P], f32)
                nc.vector.tensor_tensor(
                    out=st[:pp, :], in0=tab_ps[:pp, :], in1=bias_bc[:pp, :], op=OP.add
                )
                nc.sync.dma_start(out=tbl[r0 : r0 + pp, :], in_=st[:pp, :])

        # ---- upfront: s_src gather + streams ----
        uix = const.tile([P, NW], i32)
        nc.sync.dma_start(out=uix[:], in_=uidx_d[:])
        ssrc = const.tile([P, NW, L], f32)
        for w in range(NW):
            nc.gpsimd.indirect_dma_start(
                out=ssrc[:, w, :],
                out_offset=None,
                in_=tbl[:, :],
                in_offset=IOOA(ap=uix[:, w : w + 1], axis=0),
                element_offset=D_EM + L,
            )
        didx_sb = const.tile([P, NW * T_w], i32)
        nc.sync.dma_start(out=didx_sb[:], in_=didx_d[:])
        rankc_sb = const.tile([P, NW * T_w], f32)
        nc.sync.dma_start(out=rankc_sb[:], in_=rankc_d[:])

        # ---- edge phase ----
        with (
            tc.tile_pool(name="g", bufs=2) as gpool,
            tc.tile_pool(name="stb", bufs=2) as stbp,
            tc.tile_pool(name="st", bufs=2) as stp,
            tc.tile_pool(name="s", bufs=2) as sp,
            tc.tile_pool(name="z", bufs=2) as zp,
            tc.tile_pool(name="rev", bufs=2) as revp,
            tc.tile_pool(name="onorm", bufs=2) as onp,
            tc.tile_pool(name="psZ", bufs=2, space="PSUM") as psZ,
            tc.tile_pool(name="psU", bufs=2, space="PSUM") as psU,
        ):
            for w in range(NW):
                U_ps = psU.tile([P, 4 * D_EM + L], f32, space="PSUM")
                for (k0, tcw) in chunks:
                    c0 = w * T_w + k0
                    G = gpool.tile([P, TC, P], f32)
                    for i in range(tcw):
                        nc.gpsimd.indirect_dma_start(
                            out=G[:, i, :],
                            out_offset=None,
                            in_=tbl[:, :],
                            in_offset=IOOA(
                                ap=didx_sb[:, c0 + i : c0 + i + 1], axis=0
                            ),
                        )
                    stbits = stbp.tile([P, TC, P], u8)
                    nc.sync.dma_start(
                        out=stbits[:, :tcw, :],
                        in_=stb_d[:, c0 * P : (c0 + tcw) * P],
                    )
                    ST = stp.tile([P, TC, P], f32)
                    nc.scalar.copy(out=ST[:, :tcw, :], in_=stbits[:, :tcw, :])
                    S = sp.tile([P, TC, P], f32)
                    nc.vector.tensor_tensor(
                        out=S[:, :tcw, :],
                        in0=rankc_sb[:, c0 : c0 + tcw]
                        .unsqueeze(2)
                        .to_broadcast([P, tcw, P]),
                        in1=iota[:].unsqueeze(1).to_broadcast([P, tcw, P]),
                        op=OP.is_equal,
                    )
                    se_ps = psZ.tile([P, TC, L], f32, space="PSUM")
                    for i in range(tcw):
                        nc.tensor.matmul(
                            out=se_ps[:, i, :],
                            lhsT=ST[:, i, :],
                            rhs=ssrc[:, w, :],
                            start=True,
                            stop=True,
                        )
                    zt = zp.tile([P, TC, L], f32)
                    nc.vector.tensor_tensor(
                        out=zt[:, :tcw, :],
                        in0=se_ps[:, :tcw, :],
                        in1=G[:, :tcw, D_EM : D_EM + L],
                        op=OP.add,
                    )
                    sg = zp.tile([P, TC, L], f32)
                    nc.scalar.activation(
                        out=sg[:, :tcw, :], in_=zt[:, :tcw, :], func=AF.Sigmoid
                    )
                    Rev = revp.tile([P, TC, 4 * D_EM + L], f32)
                    nc.scalar.activation(
                        out=Rev[:, :tcw, 4 * D_EM : 4 * D_EM + L],
                        in_=sg[:, :tcw, :],
                        func=AF.Exp,
                    )
                    nc.vector.tensor_tensor(
                        out=Rev[:, :tcw, 0 : 4 * D_EM].rearrange(
                            "p t (l d) -> p t l d", l=L
                        ),
                        in0=G[:, :tcw, 0:D_EM]
                        .unsqueeze(2)
                        .to_broadcast([P, tcw, L, D_EM]),
                        in1=Rev[:, :tcw, 4 * D_EM : 4 * D_EM + L]
                        .unsqueeze(3)
                        .to_broadcast([P, tcw, L, D_EM]),
                        op=OP.mult,
                    )
                    for i in range(tcw):
                        nc.tensor.matmul(
                            out=U_ps[:, :],
                            lhsT=S[:, i, :],
                            rhs=Rev[:, i, :],
                            start=(k0 == 0 and i == 0),
                            stop=(k0 + tcw == T_w and i == tcw - 1),
                        )
                dn = onp.tile([P, L], f32)
                nc.vector.tensor_scalar(
                    out=dn[:],
                    in0=U_ps[:, 4 * D_EM : 4 * D_EM + L],
                    scalar1=1e-30,
                    scalar2=None,
                    op0=OP.add,
                )
                dnr = onp.tile([P, L], f32)
                nc.vector.reciprocal(out=dnr[:], in_=dn[:])
                ot = onp.tile([P, 4 * D_EM], f32)
                nc.vector.tensor_tensor(
                    out=ot[:].rearrange("p (l d) -> p l d", l=L),
                    in0=U_ps[:, 0 : 4 * D_EM].rearrange("p (l d) -> p l d", l=L),
                    in1=dnr[:].unsqueeze(2).to_broadcast([P, L, D_EM]),
                    op=OP.mult,
                )
                nc.sync.dma_start(out=out_d[w * P : (w + 1) * P, :], in_=ot[:])

    if split_waits:
        _split_multi_waits(nc)
    return nc


# ----------------------------------------------------------------------------
# public entry point
# ----------------------------------------------------------------------------

_NC_CACHE = {}


def _get_nc(N, NW, T_w, TC=16):
    key = (N, NW, T_w, TC)
    if key not in _NC_CACHE:
        _NC_CACHE[key] = _build_nc(N, NW, T_w, TC)
    return _NC_CACHE[key]


def _make_in_maps(x, W_lin, b_lin, att_w, att_b, emb_w, emb_b, per_core, n_cores):
    x = np.ascontiguousarray(np.asarray(x, np.float32))
    shared = dict(
        x=x,
        wl=np.ascontiguousarray(np.asarray(W_lin, np.float32)),
        aw=np.ascontiguousarray(np.asarray(att_w, np.float32)),
        ew=np.ascontiguousarray(np.asarray(emb_w, np.float32)),
        bl=np.ascontiguousarray(np.asarray(b_lin, np.float32).reshape(-1, 1)),
        embb=np.ascontiguousarray(np.asarray(emb_b, np.float32).reshape(1, -1)),
        attb=np.ascontiguousarray(np.asarray(att_b, np.float32).reshape(1, -1)),
        iota_mat=np.broadcast_to(
            np.arange(P, dtype=np.float32), (P, P)
        ).copy(),
        descale=(1.0 / (1 << (np.arange(P) // 16))).astype(np.float32).reshape(P, 1),
    )
    in_maps = []
    for c in range(n_cores):
        m = dict(shared)
        m["didx"] = per_core[c]["didx"]
        m["rankc"] = per_core[c]["rankc"]
        m["stb"] = per_core[c]["stb"]
        m["uidx"] = per_core[c]["uidx"]
        in_maps.append(m)
    return in_maps


def kernel(x, src, dst, W_lin, b_lin, att_w, att_b, emb_w, emb_b):
    from concourse.bass_utils import run_bass_kernel_spmd

    x = np.asarray(x)
    N = x.shape[0]
    cfg, per_core = _host_prep(src, dst, N, CORES)
    nc = _get_nc(N, cfg["NW"], cfg["T_w"])
    in_maps = _make_in_maps(
        x, W_lin, b_lin, att_w, att_b, emb_w, emb_b, per_core, CORES
    )
    res = run_bass_kernel_spmd(nc, in_maps, list(range(CORES)))
    out = np.zeros((N, 4 * D_EM), np.float32)
    NV = cfg["NV"]
    for c in range(CORES):
        K = per_core[c]["K"]
        u = per_core[c]["u"]
        out[c * NV + u] = res.results[c]["out"][:K]
    return out


# revision 13
# speedup vs baseline: 1.2202x; 1.2202x over previous
"""Bass/Trainium2 kernel for nn_DisentangleLayer (FactorGCN-style GNN layer).

Math (per reference):
  h    = x @ W_lin + b_lin                    [N, 256]
  h_em = x @ emb_w + emb_b                    [N, 64]
  s_src = h @ a_src.T ; s_dst = h @ a_dst.T   [N, 4]    (att_w = [a_src | a_dst])
  e    = sigmoid(s_src[src] + s_dst[dst] + att_b)       [E, 4]
  ev   = exp(e - max(e))     (max subtraction cancels in the normalized
                              ratio below; sigmoid output is bounded so no
                              overflow risk -> we skip it)
  denom = segsum_src(ev)                       [N, 4]
  out[n, 64l:64l+64] = segsum_src(ev_l * h_em[dst]) / denom[n, l]

Strategy:
  * Host shards edges by src-range across 8 cores (each core owns 12500
    nodes' outputs; no cross-core reduction needed).
  * Per core, edges are sorted by src and mapped to dense ranks; ranks are
    grouped into 128-node windows; each window's edge list is padded to a
    fixed number of 128-edge tiles (uniform across cores -> one compiled
    program).
  * Device: phase A computes a packed per-node table
    [h_em(64) | s_dst+att_b(4) | s_src(4) | pad] (f32, 512B rows) for all
    N nodes (replicated per core).  Edge phase gathers table rows by dst
    via indirect DMA, forms per-tile one-hot matrices S (edges x ranks)
    and S^T on-chip, and uses TensorE matmuls for the per-window segment
    sums of [ev*h_em | ev]; normalization by denom happens per window.
"""

import math
import numpy as np
from contextlib import ExitStack

P = 128
CORES = 8
IN_F = 256
D_EM = 64
L = 4

_PATCHED = False


def _apply_tile_patch():
    """walrus in this env rejects >1 sem-wait on one instruction; split the
    TileContext exit-drain waits across single-wait nops."""
    global _PATCHED
    if _PATCHED:
        return
    _PATCHED = True
    import concourse.tile as tile_mod
    import concourse.mybir as mybir
    from concourse.vector_clock import ScopedClock

    def _drain_and_barrier(self, tick_clock, wait_clock):
        nop = self.nc.sync.nop()
        wait_clock.add_sem_waits(nop.ins, ScopedClock({None: tick_clock.global_clock}))
        si = nop.ins.sync_info
        waits = list(si.on_wait) if si is not None else []
        if len(waits) > 1:
            si.on_wait = waits[:1]
            nop.ins.sync_info = si
            for i in range(1, len(waits)):
                extra = self.nc.sync.nop()
                extra.ins.sync_info = mybir.SyncInfo(
                    on_wait=waits[i : i + 1], on_update=[]
                )
        self.nc.sync.drain()
        self.nc.all_engine_barrier()
        assert self.sems is not None
        popped = self.nc._tile_sem_poison_stack.pop()
        assert popped is self._sem_poison
        self.nc.clear_and_free_semaphores(list(self.sems.allocated().values()))
        self.nc.all_engine_barrier()

    tile_mod.TileContext._drain_and_barrier = _drain_and_barrier


# ----------------------------------------------------------------------------
# host-side sharding / stream building
# ----------------------------------------------------------------------------

def _host_prep(src, dst, n_nodes, n_cores):
    """Shard edges by src range, sort by src, build per-core device streams.

    Returns (cfg, per_core) where per_core[c] is a dict of numpy arrays and
    cfg holds the uniform shape parameters.
    """
    NV = n_nodes // n_cores
    NW = (NV + P - 1) // P
    src = np.asarray(src)
    dst = np.asarray(dst)

    cores = []
    for c in range(n_cores):
        lo = c * NV
        sel = (src >= lo) & (src < lo + NV)
        es = src[sel].astype(np.int64) - lo
        ed = dst[sel].astype(np.int64)
        order = np.argsort(es, kind="stable")
        es = es[order]
        ed = ed[order]
        u, counts = np.unique(es, return_counts=True)
        K = len(u)
        ranks = np.repeat(np.arange(K, dtype=np.int64), counts)
        w = ranks // P
        cnt_w = np.bincount(w, minlength=NW)
        cores.append((ed, u, K, ranks, w, cnt_w))

    T_w = 1
    for (_, _, _, _, _, cnt_w) in cores:
        T_w = max(T_w, int(math.ceil(cnt_w.max() / P)))

    per_core = []
    for c in range(n_cores):
        ed, u, K, ranks, w, cnt_w = cores[c]
        lo = c * NV
        nslot = T_w * P
        slot_rank = np.full((NW, nslot), -1.0, np.float32)
        slot_dst = np.zeros((NW, nslot), np.int32)
        offs = np.concatenate([[0], np.cumsum(cnt_w)])
        pos = np.arange(len(ed)) - offs[w]
        slot_rank[w, pos] = (ranks % P).astype(np.float32)
        slot_dst[w, pos] = ed.astype(np.int32)

        # device layouts: [128, NW*T_w] with slot (w, i, p) -> [p, w*T_w + i]
        rank_col = (
            slot_rank.reshape(NW, T_w, P).transpose(2, 0, 1).reshape(P, NW * T_w)
        )
        didx = slot_dst.reshape(NW, T_w, P).transpose(2, 0, 1).reshape(P, NW * T_w)

        # ST one-hot bytes: stb[n, (w, i, e)] == 1 iff rank of slot
        # (w, i, e) == n.   (e is the partition index of the edge.)
        stb = np.zeros((P, NW, T_w, P), np.uint8)
        sr = slot_rank.reshape(NW, T_w, P)
        wv, iv, evi = np.nonzero(sr >= 0)
        nv = sr[wv, iv, evi].astype(np.int64)
        stb[nv, wv, iv, evi] = 1
        stb = stb.reshape(P, NW * T_w * P)

        u_pad = np.zeros(NW * P, np.int32)
        u_pad[:K] = (u + lo).astype(np.int32)
        uidx = u_pad.reshape(NW, P).T.copy()  # [128, NW]

        per_core.append(
            dict(didx=didx, rankc=rank_col, stb=stb, uidx=uidx, u=u, K=K)
        )

    cfg = dict(NV=NV, NW=NW, T_w=T_w)
    return cfg, per_core


# ----------------------------------------------------------------------------
# device program
# ----------------------------------------------------------------------------

def _split_multi_waits(nc):
    """This env's walrus accepts at most ONE sync-wait command per
    instruction.  Move extra waits onto single-wait nops inserted just
    before the instruction on the same engine (same sequencer => identical
    semantics)."""
    import concourse.mybir as mybir

    cnt = 0
    for f in nc.m.functions:
        for blk in f.blocks:
            insts = blk.instructions
            out = []
            changed = False
            for ins in insts:
                si = ins.sync_info
                waits = list(si.on_wait) if si is not None else []
                if len(waits) > 1:
                    changed = True
                    for w in waits[:-1]:
                        cnt += 1
                        nop = mybir.InstNoOp(
                            name=f"wsplit_{cnt}", ins=[], outs=[]
                        )
                        nop.engine = ins.engine
                        nop.sync_info = mybir.SyncInfo(on_wait=[w], on_update=[])
                        out.append(nop)
                    si.on_wait = waits[-1:]
                    ins.sync_info = si
                out.append(ins)
            if changed:
                blk.instructions = out
    return cnt


def _build_nc(N, NW, T_w, TC=16, split_waits=True):
    _apply_tile_patch()
    import concourse.bass as bass
    import concourse.mybir as mybir
    import concourse.tile as tile
    from concourse.masks import make_identity

    f32 = mybir.dt.float32
    i32 = mybir.dt.int32
    u8 = mybir.dt.uint8
    AF = mybir.ActivationFunctionType
    OP = mybir.AluOpType
    IOOA = bass.IndirectOffsetOnAxis

    nc = bass.Bass()
    x_d = nc.declare_dram_parameter("x", [N, IN_F], f32, isOutput=False)
    wl_d = nc.declare_dram_parameter("wl", [IN_F, IN_F], f32, isOutput=False)
    aw_d = nc.declare_dram_parameter("aw", [L, 2 * IN_F], f32, isOutput=False)
    ew_d = nc.declare_dram_parameter("ew", [IN_F, D_EM], f32, isOutput=False)
    bl_d = nc.declare_dram_parameter("bl", [IN_F, 1], f32, isOutput=False)
    embb_d = nc.declare_dram_parameter("embb", [1, D_EM], f32, isOutput=False)
    attb_d = nc.declare_dram_parameter("attb", [1, L], f32, isOutput=False)
    didx_d = nc.declare_dram_parameter("didx", [P, NW * T_w], i32, isOutput=False)
    rankc_d = nc.declare_dram_parameter("rankc", [P, NW * T_w], f32, isOutput=False)
    stb_d = nc.declare_dram_parameter("stb", [P, NW * T_w * P], u8, isOutput=False)
    uidx_d = nc.declare_dram_parameter("uidx", [P, NW], i32, isOutput=False)
    iota_d = nc.declare_dram_parameter("iota_mat", [P, P], f32, isOutput=False)
    descale_d = nc.declare_dram_parameter("descale", [P, 1], f32, isOutput=False)
    out_d = nc.declare_dram_parameter("out", [NW * P, 4 * D_EM], f32, isOutput=True)

    tbl = nc.dram_tensor("tbl", [N, P], f32)  # [h_em(64)|s_dst+attb(4)|s_src(4)|0]

    ntileA = (N + P - 1) // P
    chunks = []
    k0 = 0
    while k0 < T_w:
        chunks.append((k0, min(TC, T_w - k0)))
        k0 += TC

    with ExitStack() as ctx:
        tc = ctx.enter_context(tile.TileContext(nc))
        const = ctx.enter_context(tc.tile_pool(name="const", bufs=1))

        ident = const.tile([P, P], f32)
        make_identity(nc, ident[:])
        iota = const.tile([P, P], f32)
        nc.sync.dma_start(out=iota[:], in_=iota_d[:])
        descale = const.tile([P, 1], f32)
        nc.sync.dma_start(out=descale[:], in_=descale_d[:])

        # ---- fold weights: Wp[ic] = [emb_w | W@a_dst.T | W@a_src.T | 0] ----
        WT = [[const.tile([P, P], f32, name=f"WT_{j}_{i}") for i in range(2)] for j in range(2)]
        adT = [const.tile([P, L], f32, name=f"adT_{j}") for j in range(2)]
        asT = [const.tile([P, L], f32, name=f"asT_{j}") for j in range(2)]
        blT = [const.tile([P, 1], f32, name=f"blT_{j}") for j in range(2)]
        Wp = [const.tile([P, P], f32, name=f"Wp_{i}") for i in range(2)]
        bias_row = const.tile([1, P], f32)
        ones1 = const.tile([1, P], f32)
        bias_bc = const.tile([P, P], f32)
        attb_sb = const.tile([1, L], f32)

        with (
            tc.tile_pool(name="setup_sb", bufs=2) as ssb,
            tc.tile_pool(name="setup_ps", bufs=2, space="PSUM") as sps,
        ):
            for jc in range(2):
                nc.sync.dma_start(
                    out=adT[jc][:],
                    in_=aw_d[:, IN_F + jc * P : IN_F + (jc + 1) * P].transpose([1, 0]),
                )
                nc.sync.dma_start(
                    out=asT[jc][:],
                    in_=aw_d[:, jc * P : (jc + 1) * P].transpose([1, 0]),
                )
                nc.sync.dma_start(out=blT[jc][:], in_=bl_d[jc * P : (jc + 1) * P, :])
                for ic in range(2):
                    wt = ssb.tile([P, P], f32)
                    nc.sync.dma_start(
                        out=wt[:],
                        in_=wl_d[ic * P : (ic + 1) * P, jc * P : (jc + 1) * P],
                    )
                    tp = sps.tile([P, P], f32, space="PSUM")
                    nc.tensor.transpose(out=tp[:], in_=wt[:], identity=ident[:])
                    nc.vector.tensor_copy(out=WT[jc][ic][:], in_=tp[:])

            for ic in range(2):
                nc.gpsimd.memset(Wp[ic][:], 0)
                nc.sync.dma_start(
                    out=Wp[ic][:, 0:D_EM], in_=ew_d[ic * P : (ic + 1) * P, :]
                )
                wd_ps = sps.tile([P, 2 * L], f32, space="PSUM")
                for t, rhs_t in ((0, adT), (1, asT)):
                    for jc in range(2):
                        nc.tensor.matmul(
                            out=wd_ps[:, t * L : (t + 1) * L],
                            lhsT=WT[jc][ic][:],
                            rhs=rhs_t[jc][:],
                            start=(jc == 0),
                            stop=(jc == 1),
                        )
                nc.vector.tensor_copy(
                    out=Wp[ic][:, D_EM : D_EM + 2 * L], in_=wd_ps[:]
                )

            bias_ps = sps.tile([1, 2 * L], f32, space="PSUM")
            for t, rhs_t in ((0, adT), (1, asT)):
                for jc in range(2):
                    nc.tensor.matmul(
                        out=bias_ps[:, t * L : (t + 1) * L],
                        lhsT=blT[jc][:],
                        rhs=rhs_t[jc][:],
                        start=(jc == 0),
                        stop=(jc == 1),
                    )
            nc.gpsimd.memset(bias_row[:], 0)
            nc.sync.dma_start(out=bias_row[:, 0:D_EM], in_=embb_d[:])
            nc.sync.dma_start(out=attb_sb[:], in_=attb_d[:])
            nc.vector.tensor_tensor(
                out=bias_row[:, D_EM : D_EM + L],
                in0=bias_ps[:, 0:L],
                in1=attb_sb[:],
                op=OP.add,
            )
            nc.vector.tensor_copy(
                out=bias_row[:, D_EM + L : D_EM + 2 * L], in_=bias_ps[:, L : 2 * L]
            )
            # broadcast bias_row across partitions via K=1 matmul
            nc.gpsimd.memset(ones1[:], 1.0)
            bb_ps = sps.tile([P, P], f32, space="PSUM")
            nc.tensor.matmul(
                out=bb_ps[:], lhsT=ones1[:], rhs=bias_row[:], start=True, stop=True
            )
            nc.vector.tensor_copy(out=bias_bc[:], in_=bb_ps[:])

        # ---- phase A: build tbl[N, 128] ----
        with (
            tc.tile_pool(name="xa", bufs=3) as xa,
            tc.tile_pool(name="xt", bufs=3) as xtp,
            tc.tile_pool(name="stg", bufs=3) as stg,
            tc.tile_pool(name="psT", bufs=2, space="PSUM") as psT,
            tc.tile_pool(name="psM", bufs=2, space="PSUM") as psM,
        ):
            for i in range(ntileA):
                r0 = i * P
                pp = min(P, N - r0)
                xt = xa.tile([P, IN_F], f32)
                nc.sync.dma_start(out=xt[:pp, :], in_=x_d[r0 : r0 + pp, :])
                xTs = []
                for jc in range(2):
                    tp = psT.tile([P, P], f32, space="PSUM")
                    nc.tensor.transpose(
                        out=tp[:, :pp],
                        in_=xt[:pp, jc * P : (jc + 1) * P],
                        identity=ident[:pp, :pp],
                    )
                    xT = xtp.tile([P, P], f32)
                    nc.scalar.copy(out=xT[:, :pp], in_=tp[:, :pp])
                    xTs.append(xT)
                tab_ps = psM.tile([P, P], f32, space="PSUM")
                for jc in range(2):
                    nc.tensor.matmul(
                        out=tab_ps[:pp, :],
                        lhsT=xTs[jc][:, :pp],
                        rhs=Wp[jc][:],
                        start=(jc == 0),
                        stop=(jc == 1),
                    )
                st = stg.tile([P, P], f32)
                nc.vector.tensor_tensor(
                    out=st[:pp, :], in0=tab_ps[:pp, :], in1=bias_bc[:pp, :], op=OP.add
                )
                nc.sync.dma_start(out=tbl[r0 : r0 + pp, :], in_=st[:pp, :])

        # ---- upfront: s_src gather + streams ----
        uix = const.tile([P, NW], i32)
        nc.sync.dma_start(out=uix[:], in_=uidx_d[:])
        ssrc = const.tile([P, NW, L], f32)
        for w in range(NW):
            nc.gpsimd.indirect_dma_start(
                out=ssrc[:, w, :],
                out_offset=None,
                in_=tbl[:, :],
                in_offset=IOOA(ap=uix[:, w : w + 1], axis=0),
                element_offset=D_EM + L,
            )
        didx_sb = const.tile([P, NW * T_w], i32)
        nc.sync.dma_start(out=didx_sb[:], in_=didx_d[:])
        rankc_sb = const.tile([P, NW * T_w], f32)
        nc.sync.dma_start(out=rankc_sb[:], in_=rankc_d[:])

        # ---- edge phase ----
        with (
            tc.tile_pool(name="g", bufs=3) as gpool,
            tc.tile_pool(name="stb", bufs=3) as stbp,
            tc.tile_pool(name="st", bufs=3) as stp,
            tc.tile_pool(name="s", bufs=3) as sp,
            tc.tile_pool(name="z", bufs=4) as zp,
            tc.tile_pool(name="rev", bufs=2) as revp,
            tc.tile_pool(name="onorm", bufs=2) as onp,
            tc.tile_pool(name="psZ", bufs=3, space="PSUM") as psZ,
            tc.tile_pool(name="psU", bufs=2, space="PSUM") as psU,
        ):
            for w in range(NW):
                U_ps = psU.tile([P, 4 * D_EM + L], f32, space="PSUM")
                for (k0, tcw) in chunks:
                    c0 = w * T_w + k0
                    G = gpool.tile([P, TC, P], f32)
                    for i in range(tcw):
                        nc.gpsimd.indirect_dma_start(
                            out=G[:, i, :],
                            out_offset=None,
                            in_=tbl[:, :],
                            in_offset=IOOA(
                                ap=didx_sb[:, c0 + i : c0 + i + 1], axis=0
                            ),
                        )
                    stbits = stbp.tile([P, TC, P], u8)
                    nc.sync.dma_start(
                        out=stbits[:, :tcw, :],
                        in_=stb_d[:, c0 * P : (c0 + tcw) * P],
                    )
                    ST = stp.tile([P, TC, P], f32)
                    nc.scalar.copy(out=ST[:, :tcw, :], in_=stbits[:, :tcw, :])
                    S = sp.tile([P, TC, P], f32)
                    nc.vector.tensor_tensor(
                        out=S[:, :tcw, :],
                        in0=rankc_sb[:, c0 : c0 + tcw]
                        .unsqueeze(2)
                        .to_broadcast([P, tcw, P]),
                        in1=iota[:].unsqueeze(1).to_broadcast([P, tcw, P]),
                        op=OP.is_equal,
                    )
                    se_ps = psZ.tile([P, TC, L], f32, space="PSUM")
                    for i in range(tcw):
                        nc.tensor.matmul(
                            out=se_ps[:, i, :],
                            lhsT=ST[:, i, :],
                            rhs=ssrc[:, w, :],
                            start=True,
                            stop=True,
                        )
                    zt = zp.tile([P, TC, L], f32)
                    nc.vector.tensor_tensor(
                        out=zt[:, :tcw, :],
                        in0=se_ps[:, :tcw, :],
                        in1=G[:, :tcw, D_EM : D_EM + L],
                        op=OP.add,
                    )
                    sg = zp.tile([P, TC, L], f32)
                    nc.scalar.activation(
                        out=sg[:, :tcw, :], in_=zt[:, :tcw, :], func=AF.Sigmoid
                    )
                    Rev = revp.tile([P, TC, 4 * D_EM + L], f32)
                    nc.scalar.activation(
                        out=Rev[:, :tcw, 4 * D_EM : 4 * D_EM + L],
                        in_=sg[:, :tcw, :],
                        func=AF.Exp,
                    )
                    nc.vector.tensor_tensor(
                        out=Rev[:, :tcw, 0 : 4 * D_EM].rearrange(
                            "p t (l d) -> p t l d", l=L
                        ),
                        in0=G[:, :tcw, 0:D_EM]
                        .unsqueeze(2)
                        .to_broadcast([P, tcw, L, D_EM]),
                        in1=Rev[:, :tcw, 4 * D_EM : 4 * D_EM + L]
                        .unsqueeze(3)
                        .to_broadcast([P, tcw, L, D_EM]),
                        op=OP.mult,
                    )
                    for i in range(tcw):
                        nc.tensor.matmul(
                            out=U_ps[:, :],
                            lhsT=S[:, i, :],
                            rhs=Rev[:, i, :],
                            start=(k0 == 0 and i == 0),
                            stop=(k0 + tcw == T_w and i == tcw - 1),
                        )
                dn = onp.tile([P, L], f32)
                nc.vector.tensor_scalar(
                    out=dn[:],
                    in0=U_ps[:, 4 * D_EM : 4 * D_EM + L],
                    scalar1=1e-30,
                    scalar2=None,
                    op0=OP.add,
                )
                dnr = onp.tile([P, L], f32)
                nc.vector.reciprocal(out=dnr[:], in_=dn[:])
                ot = onp.tile([P, 4 * D_EM], f32)
                nc.vector.tensor_tensor(
                    out=ot[:].rearrange("p (l d) -> p l d", l=L),
                    in0=U_ps[:, 0 : 4 * D_EM].rearrange("p (l d) -> p l d", l=L),
                    in1=dnr[:].unsqueeze(2).to_broadcast([P, L, D_EM]),
                    op=OP.mult,
                )
                nc.sync.dma_start(out=out_d[w * P : (w + 1) * P, :], in_=ot[:])

    if split_waits:
        _split_multi_waits(nc)
    return nc


# ----------------------------------------------------------------------------
# public entry point
# ----------------------------------------------------------------------------

_NC_CACHE = {}


def _get_nc(N, NW, T_w, TC=16):
    key = (N, NW, T_w, TC)
    if key not in _NC_CACHE:
        _NC_CACHE[key] = _build_nc(N, NW, T_w, TC)
    return _NC_CACHE[key]


def _make_in_maps(x, W_lin, b_lin, att_w, att_b, emb_w, emb_b, per_core, n_cores):
    x = np.ascontiguousarray(np.asarray(x, np.float32))
    shared = dict(
        x=x,
        wl=np.ascontiguousarray(np.asarray(W_lin, np.float32)),
        aw=np.ascontiguousarray(np.asarray(att_w, np.float32)),
        ew=np.ascontiguousarray(np.asarray(emb_w, np.float32)),
        bl=np.ascontiguousarray(np.asarray(b_lin, np.float32).reshape(-1, 1)),
        embb=np.ascontiguousarray(np.asarray(emb_b, np.float32).reshape(1, -1)),
        attb=np.ascontiguousarray(np.asarray(att_b, np.float32).reshape(1, -1)),
        iota_mat=np.broadcast_to(
            np.arange(P, dtype=np.float32), (P, P)
        ).copy(),
        descale=(1.0 / (1 << (np.arange(P) // 16))).astype(np.float32).reshape(P, 1),
    )
    in_maps = []
    for c in range(n_cores):
        m = dict(shared)
        m["didx"] = per_core[c]["didx"]
        m["rankc"] = per_core[c]["rankc"]
        m["stb"] = per_core[c]["stb"]
        m["uidx"] = per_core[c]["uidx"]
        in_maps.append(m)
    return in_maps


def kernel(x, src, dst, W_lin, b_lin, att_w, att_b, emb_w, emb_b):
    from concourse.bass_utils import run_bass_kernel_spmd

    x = np.asarray(x)
    N = x.shape[0]
    cfg, per_core = _host_prep(src, dst, N, CORES)
    nc = _get_nc(N, cfg["NW"], cfg["T_w"])
    in_maps = _make_in_maps(
        x, W_lin, b_lin, att_w, att_b, emb_w, emb_b, per_core, CORES
    )
    res = run_bass_kernel_spmd(nc, in_maps, list(range(CORES)))
    out = np.zeros((N, 4 * D_EM), np.float32)
    NV = cfg["NV"]
    for c in range(CORES):
        K = per_core[c]["K"]
        u = per_core[c]["u"]
        out[c * NV + u] = res.results[c]["out"][:K]
    return out
